# revision 46
# baseline (speedup 1.0000x reference)
"""Distributed Trainium2 kernel for the GNN message-passing model.

Self-contained: host-side structural prep (sharding, edge sort, index
remap) + Bass/Tile SPMD kernel across 8 NeuronCores.

Math (see reference):
  logits = MLP(x1); m = 0.15 + 0.55*onehot(argmax(logits))
  r1 = (m@W1.sum(-1))*x2 + m@bp1
  g1 = relu(Dh A Dh (r1@gcn1_w) + gcn1_b); g1 = (m@W12)*g1 + 2e-4*(r1@W13)
  r2 = (m@W2.sum(-1))*g1 + m@bp2
  g2 = relu(Dh A Dh (r2@gcn2_w) + gcn2_b)
  out = log_softmax(g2@fc_w + fc_b)
where Dh = diag(deg^-1/2), deg = in-degree over dst.

Distribution: nodes sharded contiguously over 8 cores. Per GCN layer the
scaled features h' = Dh*h are AllGathered in fp8 (in node chunks, so comm
overlaps the producer pipeline); each core gathers h'[src] for edges
whose dst it owns via indirect DMA and scatter-reduces them with
one-hot matmuls on the TensorEngine (PSUM accumulation per dst block).
The router MLP and the r1->(gcn1_w|W13) matmuls run in fp8 DoubleRow
(2 fp8 MACs/cell/cycle); scatter masks/tables and collective payloads
are fp8e4m3 with power-of-two scaling. Layer-1 scatter runs in three
rounds keyed to AllGather chunk arrival so scatter work overlaps the
front; partial aggregates accumulate in SBUF (bf16).
"""

import numpy as np

P = 128
TAU_HI = 0.7
TAU_LO = 0.15   # (1-0.7)/2
S1 = 512.0      # fp8 scale for layer-1 h' table
S2 = 32768.0    # fp8 scale for layer-2 h' table
SZ = 524288.0   # fp8 scale for the z (W13) term
SW = 1024.0     # fp8 scale for mlp_w1/mlp_w2/gcn1_w
SW3 = 2048.0    # fp8 scale for mlp_w3
SH = 32.0       # fp8 scale for MLP hidden activations + r1
SWZ = float(2 ** 22)  # fp8 scale for W13*2e-4 inside the fused rhs


class _Cfg:
    def __init__(self, N, E, F1=768, H=512, G1=256, G2=32, FOUT=40, C=7):
        self.NC = 8
        self.N = N
        self.E = E
        self.NLOC_RAW = N // self.NC
        self.NB = -(-self.NLOC_RAW // P)          # node blocks per core
        self.NLOC = self.NB * P
        assert self.NB % C == 0, (self.NB, C)
        self.C = C                                 # allgather chunks
        self.BPC = self.NB // C                    # blocks per chunk
        self.CH = self.BPC * P                     # chunk nodes
        self.TR = self.NC * self.NLOC              # gathered table rows
        self.CHR = self.NC * self.CH               # rows per chunk in table
        self.F1, self.H, self.G1, self.G2, self.FOUT = F1, H, G1, G2, FOUT
        self.KF1 = F1 // P                         # 6 k-tiles
        self.KH = H // P                           # 4
        self.KG1 = G1 // P                         # 2
        self.SPL1 = [1, 5]                         # L1 round chunk splits
        self.SPL2 = 3                  # L2 round-A src chunks
        self.NFREE = min(448, self.CH)             # front free-dim unit
        assert self.CH % self.NFREE == 0
        self.FU = self.CH // self.NFREE            # free units per chunk


CFG_FULL = dict(N=50000, E=800000)


def _to_bf16(x):
    import ml_dtypes
    return np.asarray(x, np.float32).astype(ml_dtypes.bfloat16)


def _to_f8(x, scale=1.0):
    import ml_dtypes
    return np.clip(np.asarray(x, np.float32) * scale, -240, 240).astype(
        ml_dtypes.float8_e4m3)


def _rows_l1(v, cfg):
    """Gathered-table row for global node id v (vectorized).

    One AllGather per chunk k, each writing the [k*CHR, (k+1)*CHR) slice
    of the (conceptual) global table as a rank-major concat of that
    chunk: row = k*CHR + c*CH + off. Round tables are contiguous slices
    of this global row space (build_layout subtracts the round base)."""
    c = v // cfg.NLOC_RAW
    s = v - c * cfg.NLOC_RAW
    k = s // cfg.CH
    return k * cfg.CHR + c * cfg.CH + (s - k * cfg.CH)


_rows_l2 = _rows_l1


def host_prep(inputs, cfg):
    """Returns (in_maps, sched). sched is baked into the built graph and
    must be identical for every core (SPMD)."""
    x1 = np.asarray(inputs["x1"], np.float32)
    x2 = np.asarray(inputs["x2"], np.float32)
    ei = np.asarray(inputs["edge_index"])
    src = ei[0].astype(np.int64)
    dst = ei[1].astype(np.int64)
    N, E, NC = cfg.N, cfg.E, cfg.NC
    assert x1.shape[0] == N and src.shape[0] == E

    deg = np.bincount(dst, minlength=N).astype(np.float64)
    dinv = np.where(deg > 0, deg ** -0.5, 0.0).astype(np.float32)
    sdeg = np.sqrt(deg).astype(np.float32)  # 1/dinv where deg>0 else 0

    # ---- per-core edge partition by dst owner, sorted by dst block ----
    owner = dst // cfg.NLOC_RAW
    dloc = dst - owner * cfg.NLOC_RAW
    dblk = dloc // P
    drel_all = (dloc - dblk * P).astype(np.float32)
    rows_l1 = _rows_l1(src, cfg).astype(np.int32)
    rows_l2 = _rows_l2(src, cfg).astype(np.int32)

    def split_core(rows_all):
        per_core = []
        for c in range(NC):
            sel = np.where(owner == c)[0]
            order = np.argsort(dblk[sel], kind="stable")
            sel = sel[order]
            b_of = dblk[sel]
            bounds = np.searchsorted(b_of, np.arange(cfg.NB + 1))
            lists = []
            for b in range(cfg.NB):
                idxs = sel[bounds[b]:bounds[b + 1]]
                lists.append((rows_all[idxs], drel_all[idxs]))
            per_core.append(lists)
        return per_core

    per_core_l1 = split_core(rows_l1)
    per_core_l2 = split_core(rows_l2)

    # Uniform cross-core layouts split in rounds by src chunk
    # (round boundary = which AllGather chunks the gathers depend on).
    def build_layout(per_core, chunk_splits, pad_mult):
        bounds_k = [0] + [s * cfg.CHR for s in chunk_splits] + [cfg.C * cfg.CHR]
        layout = dict(rounds=[])
        for r in range(len(bounds_k) - 1):
            lo, hi = bounds_k[r], bounds_k[r + 1]
            cntr = np.zeros((NC, cfg.NB), np.int64)
            per_rc = []
            for c in range(NC):
                pc = []
                for b in range(cfg.NB):
                    rows, rel = per_core[c][b]
                    m = (rows >= lo) & (rows < hi)
                    pc.append((rows[m] - lo, rel[m]))
                    cntr[c, b] = int(m.sum())
                per_rc.append(pc)
            Kb = np.maximum(1, -(-cntr.max(axis=0) // P)).astype(np.int64)
            nb_round = int(Kb.sum())
            pad = (-nb_round) % pad_mult
            nb_round += pad
            b_of = np.concatenate([np.repeat(np.arange(cfg.NB), Kb),
                                   np.full(pad, cfg.NB - 1)])
            first = np.zeros(nb_round, bool)
            last = np.zeros(nb_round, bool)
            off = 0
            for b in range(cfg.NB):
                first[off] = True
                e = off + int(Kb[b])
                if b == cfg.NB - 1:
                    e = nb_round
                last[e - 1] = True
                off += int(Kb[b])
            layout["rounds"].append(dict(Kb=Kb, nblocks=nb_round, b_of=b_of,
                                         first=first, last=last,
                                         per_rc=per_rc))
        return layout

    lay1 = build_layout(per_core_l1, cfg.SPL1, 32)
    lay2 = build_layout(per_core_l2, [cfg.SPL2], 32)

    def pack_layout(layout, c):
        idxs, Ss = [], []
        for rr in layout["rounds"]:
            sbs = rr["nblocks"] // 8
            idx = np.zeros((sbs * P, 8), np.int32)
            drl = np.full((sbs * P, 8), -1.0, np.float32)
            g = 0
            for b in range(cfg.NB):
                rows, rel = rr["per_rc"][c][b]
                n = len(rows)
                nblk = int(rr["Kb"][b])
                if b == cfg.NB - 1:
                    nblk = rr["nblocks"] - g
                for j in range(nblk):
                    s, jj = g // 8, g % 8
                    e0 = j * P
                    m = min(P, max(0, n - e0))
                    if m > 0:
                        idx[s * P:s * P + m, jj] = rows[e0:e0 + m]
                        drl[s * P:s * P + m, jj] = rel[e0:e0 + m]
                    g += 1
            idxs.append(idx)
            Ss.append(_to_f8(
                (drl[:, :, None] ==
                 np.arange(P, dtype=np.float32)[None, None, :])
                .astype(np.float32).reshape(sbs * P, 8 * P)))
        return (np.concatenate(idxs, axis=0), np.concatenate(Ss, axis=0))

    sched = dict(lay1=[dict(nblocks=r["nblocks"], b_of=r["b_of"],
                            first=r["first"], last=r["last"])
                       for r in lay1["rounds"]],
                 lay2=[dict(nblocks=r["nblocks"], b_of=r["b_of"],
                            first=r["first"], last=r["last"])
                       for r in lay2["rounds"]])

    # ---- weights ----
    w1 = np.asarray(inputs["mlp_w1"], np.float32)
    w2 = np.asarray(inputs["mlp_w2"], np.float32)
    w3 = np.asarray(inputs["mlp_w3"], np.float32)
    b1 = np.asarray(inputs["mlp_b1"], np.float32)
    b2 = np.asarray(inputs["mlp_b2"], np.float32)
    b3 = np.asarray(inputs["mlp_b3"], np.float32)
    W1s = np.asarray(inputs["W1"], np.float32).sum(-1)
    W12 = np.asarray(inputs["W12"], np.float32)
    W13 = np.asarray(inputs["W13"], np.float32) * 2e-4
    bp1 = np.asarray(inputs["bp1"], np.float32)
    W2s = np.asarray(inputs["W2"], np.float32).sum(-1)
    bp2 = np.asarray(inputs["bp2"], np.float32)
    g1w = np.asarray(inputs["gcn1_w"], np.float32)
    g1b = np.asarray(inputs["gcn1_b"], np.float32)
    g2w = np.asarray(inputs["gcn2_w"], np.float32)
    g2b = np.asarray(inputs["gcn2_b"], np.float32)
    fcw = np.asarray(inputs["fc_w"], np.float32)
    fcb = np.asarray(inputs["fc_b"], np.float32)

    sched["bp1_nz"] = bool(np.any(bp1 != 0))
    sched["bp2_nz"] = bool(np.any(bp2 != 0))
    sched["g1b_nz"] = bool(np.any(g1b != 0))
    sched["g2b_nz"] = bool(np.any(g2b != 0))
    sched["fcb_nz"] = bool(np.any(fcb != 0))
    sched["b3_nz"] = bool(np.any(b3 != 0))

    def pack_lhsT(w, KT, MT, conv):
        o = np.zeros((P, KT * MT * P), np.float32)
        for k in range(KT):
            for m in range(MT):
                o[:, (k * MT + m) * P:(k * MT + m + 1) * P] = \
                    w[k * P:(k + 1) * P, m * P:(m + 1) * P]
        return conv(o)

    def pack_rhs(w, KT, F, conv):
        o = np.zeros((P, KT * F), np.float32)
        for k in range(KT):
            o[:, k * F:(k + 1) * F] = w[k * P:(k + 1) * P, :]
        return conv(o)

    def pack_k3(w, F):
        o = np.zeros((4, F), np.float32)
        o[:3] = w
        return _to_bf16(o)

    w1_p = pack_lhsT(w1 * SW, cfg.KF1, cfg.KH, _to_f8)
    w2_p = pack_lhsT(w2 * SW, cfg.KH, cfg.KH, _to_f8)
    w3_p = pack_rhs(np.pad(w3, ((0, 0), (0, 1))) * SW3, cfg.KH, 4, _to_f8)
    b1_p = b1.reshape(cfg.KH, P).T.copy() * SH  # ACT bias adds after scale
    b2_p = b2.reshape(cfg.KH, P).T.copy() * SH
    b3_p = (np.pad(b3, (0, 1)) * (SH * SW3)).reshape(1, 4).repeat(P, 0).copy()
    # fused [g1w*SW | W13*2e-4*SWZ] rhs: per k-tile 512 wide, fp8
    g1f = np.concatenate([g1w * SW, W13 * SWZ], axis=1)  # [768, 512]
    g1f_p = pack_rhs(g1f, cfg.KF1, 2 * cfg.G1, _to_f8)
    g2w_p = pack_rhs(g2w, cfg.KG1, cfg.G2, _to_bf16)
    fcw_p = _to_bf16(fcw)
    W1s_p = pack_k3(W1s, cfg.F1)
    bp1_p = pack_k3(bp1 * SH, cfg.F1)
    W12_p = pack_k3(W12, cfg.G1)
    W2s_p = pack_k3(W2s, cfg.G1)
    bp2_p = pack_k3(bp2, cfg.G1)
    g1b_p = _to_bf16(g1b.reshape(1, cfg.G1) * S1)
    g2b_p = _to_bf16(g2b.reshape(1, cfg.G2) * S2)
    fcb_p = _to_bf16(fcb.reshape(1, cfg.FOUT))

    in_maps = []
    for c in range(NC):
        lo = c * cfg.NLOC_RAW
        hi = lo + cfg.NLOC_RAW
        x1T = np.zeros((cfg.F1, cfg.NLOC), np.float32)
        x1T[:, :cfg.NLOC_RAW] = x1[lo:hi].T
        x2T = np.zeros((cfg.F1, cfg.NLOC), np.float32)
        x2T[:, :cfg.NLOC_RAW] = x2[lo:hi].T
        dinv_t = np.zeros((P, cfg.NB), np.float32)
        dinv_t.T.reshape(-1)[:cfg.NLOC_RAW] = dinv[lo:hi]
        sdeg_r = np.zeros((1, cfg.NLOC), np.float32)
        sdeg_r[0, :cfg.NLOC_RAW] = sdeg[lo:hi]

        ident_np = _to_bf16(np.eye(P, dtype=np.float32))
        idx1, Sm1 = pack_layout(lay1, c)
        idx2, Sm2 = pack_layout(lay2, c)
        im = {
            "ident": ident_np,
            "x1T": _to_f8(x1T), "x2T": _to_f8(x2T),
            "idx1": idx1, "Sm1": Sm1, "idx2": idx2, "Sm2": Sm2,
            "dh1": dinv_t * (S1 / (SH * SW)), "dl1": dinv_t * (1.0 / S1),
            "dh2": dinv_t * S2, "dl2": dinv_t * (1.0 / S2),
            "sdeg_r": _to_bf16(sdeg_r),
            "w1": w1_p, "w2": w2_p, "w3": w3_p,
            "b1": b1_p, "b2": b2_p, "b3": b3_p,
            "g1f": g1f_p, "g2w": g2w_p, "fcw": fcw_p,
            "W1s": W1s_p, "bp1": bp1_p, "W12": W12_p, "W2s": W2s_p,
            "bp2": bp2_p, "g1b": g1b_p, "g2b": g2b_p, "fcb": fcb_p,
        }
        in_maps.append(im)
    return in_maps, sched


def build(cfg, sched, debug=False):
    import concourse.bacc as bacc
    import concourse.bass as bass
    import concourse.mybir as mybir
    import concourse.tile as tile

    dt = mybir.dt
    AF = mybir.ActivationFunctionType
    OP = mybir.AluOpType
    AX = mybir.AxisListType
    DR = mybir.MatmulPerfMode.DoubleRow

    nc = bacc.Bacc("TRN2", target_bir_lowering=False, debug=debug)

    NB, C, BPC, CH, NLOC, TR, CHR = (cfg.NB, cfg.C, cfg.BPC, cfg.CH,
                                     cfg.NLOC, cfg.TR, cfg.CHR)
    F1, H, G1, G2, FOUT = cfg.F1, cfg.H, cfg.G1, cfg.G2, cfg.FOUT
    KF1, KH, KG1 = cfg.KF1, cfg.KH, cfg.KG1
    NF, FU = cfg.NFREE, cfg.FU
    L1R = sched["lay1"]            # 3 rounds
    L2A, L2B = sched["lay2"]
    SB1 = sum(r["nblocks"] for r in L1R) // 8
    SB2T = (L2A["nblocks"] + L2B["nblocks"]) // 8
    SPL1, SPL2 = cfg.SPL1, cfg.SPL2

    bf = dt.bfloat16
    f32 = dt.float32
    f8 = dt.float8e4

    dd = {}

    def din(name, shape, dtype):
        dd[name] = nc.declare_dram_parameter(name, list(shape), dtype,
                                             isOutput=False)
        return dd[name]

    x1T_d = din("x1T", [F1, NLOC], f8)
    x2T_d = din("x2T", [F1, NLOC], f8)
    idx1_d = din("idx1", [SB1 * P, 8], dt.int32)
    Sm1_d = din("Sm1", [SB1 * P, 8 * P], f8)
    idx2_d = din("idx2", [SB2T * P, 8], dt.int32)
    Sm2_d = din("Sm2", [SB2T * P, 8 * P], f8)
    dh1_d = din("dh1", [P, NB], f32)
    dl1_d = din("dl1", [P, NB], f32)
    dh2_d = din("dh2", [P, NB], f32)
    dl2_d = din("dl2", [P, NB], f32)
    sdeg_d = din("sdeg_r", [1, NLOC], bf)
    w1_d = din("w1", [P, KF1 * KH * P], f8)
    w2_d = din("w2", [P, KH * KH * P], f8)
    w3_d = din("w3", [P, KH * 4], f8)
    b1_d = din("b1", [P, KH], f32)
    b2_d = din("b2", [P, KH], f32)
    b3_d = din("b3", [P, 4], f32)
    g1f_d = din("g1f", [P, KF1 * 2 * G1], f8)
    g2w_d = din("g2w", [P, KG1 * G2], bf)
    fcw_d = din("fcw", [G2, FOUT], bf)
    W1s_d = din("W1s", [4, F1], bf)
    bp1_d = din("bp1", [4, F1], bf)
    W12_d = din("W12", [4, G1], bf)
    W2s_d = din("W2s", [4, G1], bf)
    bp2_d = din("bp2", [4, G1], bf)
    g1b_d = din("g1b", [1, G1], bf)
    g2b_d = din("g2b", [1, G2], bf)
    fcb_d = din("fcb", [1, FOUT], bf)
    ident_d = din("ident", [P, P], bf)
    out_d = nc.declare_dram_parameter("out", [NLOC, FOUT], f32, isOutput=True)
    import os
    DBG = bool(os.environ.get("K_DBG"))
    if DBG:
        dbg_n = 11
        dbg_t = nc.declare_dram_parameter("dbg_t", [dbg_n * P, G1], f8,
                                          isOutput=True)

    with tile.TileContext(nc) as tc:
        with (
            tc.tile_pool(name="const", bufs=1) as cp,
            tc.tile_pool(name="front", bufs=2) as fp,
            tc.tile_pool(name="scat", bufs=3) as sp,
            tc.tile_pool(name="fin", bufs=2) as qp,
            tc.tile_pool(name="psG", bufs=2, space="PSUM") as psG,
            tc.tile_pool(name="psH", bufs=2, space="PSUM") as psH,
            tc.tile_pool(name="psS", bufs=2, space="PSUM") as psS,
            tc.tile_pool(name="psB", bufs=1, space="PSUM") as psB,
            tc.tile_pool(name="psT", bufs=1, space="PSUM") as psT,
            tc.tile_pool(name="dram", bufs=1, space="DRAM") as dp,
        ):
            def load(dr, shape, dtype, name):
                t = cp.tile(shape, dtype, tag=name)
                nc.sync.dma_start(out=t[:, :], in_=dr[:, :])
                return t

            w1_s = load(w1_d, [P, KF1 * KH * P], f8, "w1")
            w2_s = load(w2_d, [P, KH * KH * P], f8, "w2")
            w3_s = load(w3_d, [P, KH * 4], f8, "w3")
            b1_s = load(b1_d, [P, KH], f32, "b1")
            b2_s = load(b2_d, [P, KH], f32, "b2")
            b3_s = load(b3_d, [P, 4], f32, "b3")
            g1f_s = load(g1f_d, [P, KF1 * 2 * G1], f8, "g1f")
            g2w_s = load(g2w_d, [P, KG1 * G2], bf, "g2w")
            fcw_s = load(fcw_d, [G2, FOUT], bf, "fcw")
            W1s_s = load(W1s_d, [4, F1], bf, "W1s")
            bp1_s = load(bp1_d, [4, F1], bf, "bp1")
            W12_s = load(W12_d, [4, G1], bf, "W12")
            W2s_s = load(W2s_d, [4, G1], bf, "W2s")
            bp2_s = load(bp2_d, [4, G1], bf, "bp2")
            g1b_s = load(g1b_d, [1, G1], bf, "g1b")
            g2b_s = load(g2b_d, [1, G2], bf, "g2b")
            fcb_s = load(fcb_d, [1, FOUT], bf, "fcb")
            dh1_s = load(dh1_d, [P, NB], f32, "dh1")
            dl1_s = load(dl1_d, [P, NB], f32, "dl1")
            dh2_s = load(dh2_d, [P, NB], f32, "dh2")
            dl2_s = load(dl2_d, [P, NB], f32, "dl2")
            sdeg_s = load(sdeg_d, [1, NLOC], bf, "sdeg")

            ident = load(ident_d, [P, P], bf, "ident")
            ones1 = cp.tile([1, P], bf, tag="ones1")
            nc.vector.memset(ones1[:, :], 1.0)

            mT_s = cp.tile([4, NLOC], bf, tag="mT")
            out_acc = cp.tile([P, NB * FOUT], f32, tag="oacc")
            z_s = cp.tile([P, NB * G1], f8, tag="z")
            aggA_s = cp.tile([P, NB * G1], bf, tag="aggA")
            agg2_s = cp.tile([P, NB * G2], bf, tag="agg2")

            n_r1 = [SPL1[0], SPL1[1] - SPL1[0], C - SPL1[1]]
            h1bs = [dp.tile([CH, G1], f8, tag=f"h1b{k}", name=f"h1b{k}")
                    for k in range(C)]
            h2bs = [dp.tile([CH, G2], f8, tag=f"h2b{k}", name=f"h2b{k}")
                    for k in range(C)]
            h1gs = [dp.tile([n_r1[r] * CHR, G1], f8, tag=f"h1g{r}",
                            name=f"h1g{r}")
                    for r in range(3)]
            h2gA = dp.tile([SPL2 * CHR, G2], f8, tag="h2gA")
            h2gB = dp.tile([(C - SPL2) * CHR, G2], f8, tag="h2gB")

            # ================= FRONT (per chunk) =================
            for k in range(C):
                n0 = k * CH
                x1c = fp.tile([P, KF1 * CH], f8, tag="x1c")
                nc.sync.dma_start(
                    out=x1c[:, :].rearrange("p (a n) -> p a n", n=CH),
                    in_=x1T_d[:, n0:n0 + CH].rearrange("(a p) n -> p a n", p=P))
                x2c = fp.tile([P, KF1 * CH], f8, tag="x2c", bufs=2)
                nc.sync.dma_start(
                    out=x2c[:, :].rearrange("p (a n) -> p a n", n=CH),
                    in_=x2T_d[:, n0:n0 + CH].rearrange("(a p) n -> p a n", p=P))

                x1c3 = x1c[:, :].rearrange("p (a n) -> p a n", n=CH)
                w1c4 = w1_s[:, :].rearrange("p (a m c) -> p a m c", m=KH, c=P)
                h1T = fp.tile([P, KH * CH], f8, tag="h1T", bufs=1)
                for u in range(FU):
                    for m in range(KH):
                        ps = psG.tile([P, NF], f32, tag="g")
                        for kk in range(0, KF1, 2):
                            nc.tensor.matmul(
                                ps[:, :],
                                lhsT=w1c4[:, kk:kk + 2, m:m + 1, :],
                                rhs=x1c3[:, kk:kk + 2,
                                         u * NF:u * NF + NF],
                                perf_mode=DR,
                                start=(kk == 0), stop=(kk == KF1 - 2))
                        nc.scalar.activation(
                            h1T[:, m * CH + u * NF:m * CH + u * NF + NF],
                            ps[:, :], AF.Relu, bias=b1_s[:, m:m + 1],
                            scale=SH / SW)
                h1T3 = h1T[:, :].rearrange("p (a n) -> p a n", n=CH)
                w2c4 = w2_s[:, :].rearrange("p (a m c) -> p a m c", m=KH, c=P)
                h2T = fp.tile([P, KH * CH], f8, tag="h2T", bufs=1)
                for u in range(FU):
                    for m in range(KH):
                        ps = psG.tile([P, NF], f32, tag="g")
                        for kk in range(0, KH, 2):
                            nc.tensor.matmul(
                                ps[:, :],
                                lhsT=w2c4[:, kk:kk + 2, m:m + 1, :],
                                rhs=h1T3[:, kk:kk + 2,
                                         u * NF:u * NF + NF],
                                perf_mode=DR,
                                start=(kk == 0), stop=(kk == KH - 2))
                        nc.scalar.activation(
                            h2T[:, m * CH + u * NF:m * CH + u * NF + NF],
                            ps[:, :], AF.Relu, bias=b2_s[:, m:m + 1],
                            scale=1.0 / SW)

                mmc = fp.tile([P, BPC * 3], bf, tag="mmc")
                for nb in range(BPC):
                    psl = psB.tile([P, 512], f32, tag="b")
                    for kk in range(KH):
                        nc.tensor.matmul(
                            psl[:, :4],
                            lhsT=h2T[:, kk * CH + nb * P:kk * CH + (nb + 1) * P],
                            rhs=w3_s[:, kk * 4:(kk + 1) * 4],
                            start=(kk == 0), stop=(kk == KH - 1))
                    lg = fp.tile([P, 3], f32, tag="lg")
                    if sched["b3_nz"]:
                        nc.vector.tensor_add(lg[:, :], psl[:, :3], b3_s[:, :3])
                    else:
                        nc.vector.tensor_copy(lg[:, :], psl[:, :3])
                    rmax = fp.tile([P, 1], f32, tag="rmax")
                    nc.vector.reduce_max(rmax[:, :], lg[:, :], axis=AX.X)
                    mm = fp.tile([P, 3], bf, tag="mm")
                    nc.vector.tensor_scalar(
                        mm[:, :], lg[:, :], rmax[:, :1], None, OP.is_equal)
                    nc.scalar.activation(mmc[:, nb * 3:(nb + 1) * 3],
                                         mm[:, :], AF.Copy,
                                         bias=TAU_LO, scale=TAU_HI - TAU_LO)
                for nb in range(BPC):
                    b_glob = k * BPC + nb
                    pst = psT.tile([P, P], bf, tag="t")
                    nc.tensor.transpose(pst[:3, :],
                                        mmc[:, nb * 3:(nb + 1) * 3],
                                        ident[:, :])
                    nc.vector.tensor_copy(
                        mT_s[:3, b_glob * P:(b_glob + 1) * P], pst[:3, :])

                r1T = fp.tile([P, KF1 * CH], f8, tag="r1T")
                for u in range(FU):
                    for f in range(KF1):
                        psr = psG.tile([P, NF], f32, tag="g")
                        nc.tensor.matmul(
                            psr[:, :], lhsT=W1s_s[:3, f * P:(f + 1) * P],
                            rhs=mT_s[:3, n0 + u * NF:n0 + u * NF + NF],
                            start=True, stop=True)
                        if sched["bp1_nz"]:
                            psr2 = psB.tile([P, 512], f32, tag="b")
                            nc.tensor.matmul(
                                psr2[:, :NF], lhsT=bp1_s[:3, f * P:(f + 1) * P],
                                rhs=mT_s[:3, n0 + u * NF:n0 + u * NF + NF],
                                start=True, stop=True)
                            tmp = fp.tile([P, NF], f32, tag="r1tmp")
                            nc.vector.tensor_mul(
                                tmp[:, :], psr[:, :],
                                x2c[:, f * CH + u * NF:f * CH + u * NF + NF])
                            nc.vector.scalar_tensor_tensor(
                                out=r1T[:, f * CH + u * NF:f * CH + u * NF + NF],
                                in0=tmp[:, :], scalar=SH,
                                in1=psr2[:, :NF], op0=OP.mult, op1=OP.add)
                        else:
                            nc.vector.scalar_tensor_tensor(
                                out=r1T[:, f * CH + u * NF:f * CH + u * NF + NF],
                                in0=psr[:, :], scalar=SH,
                                in1=x2c[:, f * CH + u * NF:f * CH + u * NF + NF],
                                op0=OP.mult, op1=OP.mult)

                r1T3 = r1T[:, :].rearrange("p (a n) -> p a n", n=CH)
                g1f3 = g1f_s[:, :].rearrange("p (a c) -> p a c", c=2 * G1)
                ri = 0 if k < SPL1[0] else (1 if k < SPL1[1] else 2)
                k0 = [0, SPL1[0], SPL1[1]][ri]
                for nb in range(BPC):
                    b_glob = k * BPC + nb
                    psh = psH.tile([P, 2 * G1], f32, tag="h")
                    for f in range(0, KF1, 2):
                        nc.tensor.matmul(
                            psh[:, :],
                            lhsT=r1T3[:, f:f + 2, nb * P:(nb + 1) * P],
                            rhs=g1f3[:, f:f + 2, :],
                            perf_mode=DR,
                            start=(f == 0), stop=(f == KF1 - 2))
                    h1p = fp.tile([P, G1], f8, tag="h1p")
                    nc.scalar.activation(h1p[:, :], psh[:, :G1], AF.Copy,
                                         scale=dh1_s[:, b_glob:b_glob + 1])
                    nc.scalar.dma_start(
                        out=h1bs[k][nb * P:(nb + 1) * P, :], in_=h1p[:, :])
                    nc.scalar.activation(
                        z_s[:, b_glob * G1:(b_glob + 1) * G1],
                        psh[:, G1:2 * G1], AF.Copy, scale=SZ / (SH * SWZ))

                agt = h1gs[ri][(k - k0) * CHR:(k - k0 + 1) * CHR, :]
                nc.gpsimd.collective_compute(
                    "AllGather", OP.bypass,
                    replica_groups=[list(range(cfg.NC))],
                    ins=[h1bs[k][:, :].opt()],
                    outs=[agt.opt()])

            # ================= LAYER 1 scatter (3 rounds) =================
            ps_by_b = {}

            def l1_finalize(b):
                psb = ps_by_b.pop(b)
                if sched["g1b_nz"]:
                    nc.tensor.matmul(
                        psb[:, :], lhsT=sdeg_s[:1, b * P:(b + 1) * P],
                        rhs=g1b_s[:1, :], start=False, stop=True,
                        skip_group_check=True)
                g1pre = qp.tile([P, G1], bf, tag="g1pre", bufs=3)
                nc.vector.tensor_add(g1pre[:, :], psb[:, :],
                                     aggA_s[:, b * G1:(b + 1) * G1])
                g1r = qp.tile([P, G1], bf, tag="g1r", bufs=3)
                nc.scalar.activation(g1r[:, :], g1pre[:, :], AF.Relu,
                                     scale=dl1_s[:, b:b + 1])
                psmw = psH.tile([P, 2 * G1], f32, tag="h")
                nc.tensor.matmul(psmw[:, :G1],
                                 lhsT=mT_s[:3, b * P:(b + 1) * P],
                                 rhs=W12_s[:3, :], start=True, stop=True)
                nc.tensor.matmul(psmw[:, G1:2 * G1],
                                 lhsT=mT_s[:3, b * P:(b + 1) * P],
                                 rhs=W2s_s[:3, :], start=True, stop=True)
                g1t = qp.tile([P, G1], bf, tag="g1t", bufs=3)
                nc.vector.tensor_mul(g1t[:, :], g1r[:, :], psmw[:, :G1])
                g1v = qp.tile([P, G1], bf, tag="g1v", bufs=3)
                nc.vector.scalar_tensor_tensor(
                    out=g1v[:, :], in0=z_s[:, b * G1:(b + 1) * G1],
                    scalar=1.0 / SZ, in1=g1t[:, :],
                    op0=OP.mult, op1=OP.add)
                r2 = qp.tile([P, G1], bf, tag="r2", bufs=3)
                if sched["bp2_nz"]:
                    psm3 = psB.tile([P, 512], f32, tag="b")
                    nc.tensor.matmul(psm3[:, :G1],
                                     lhsT=mT_s[:3, b * P:(b + 1) * P],
                                     rhs=bp2_s[:3, :], start=True, stop=True)
                    r2u = qp.tile([P, G1], bf, tag="r2u")
                    nc.vector.tensor_mul(r2u[:, :], g1v[:, :],
                                         psmw[:, G1:2 * G1])
                    r2v = qp.tile([P, G1], bf, tag="r2v")
                    nc.vector.tensor_add(r2v[:, :], r2u[:, :], psm3[:, :G1])
                    nc.vector.tensor_scalar(r2[:, :], r2v[:, :],
                                            dh2_s[:, b:b + 1], None, OP.mult)
                else:
                    nc.vector.scalar_tensor_tensor(
                        out=r2[:, :], in0=g1v[:, :],
                        scalar=dh2_s[:, b:b + 1],
                        in1=psmw[:, G1:2 * G1], op0=OP.mult, op1=OP.mult)
                r2T = qp.tile([P, KG1 * P], bf, tag="r2T", bufs=3)
                for f in range(KG1):
                    pst = psT.tile([P, P], bf, tag="t")
                    nc.tensor.transpose(pst[:, :], r2[:, f * P:(f + 1) * P],
                                        ident[:, :])
                    nc.vector.tensor_copy(r2T[:, f * P:(f + 1) * P],
                                          pst[:, :])
                psh2 = psB.tile([P, 512], f32, tag="b")
                for f in range(KG1):
                    nc.tensor.matmul(
                        psh2[:, :G2], lhsT=r2T[:, f * P:(f + 1) * P],
                        rhs=g2w_s[:, f * G2:(f + 1) * G2],
                        start=(f == 0), stop=(f == KG1 - 1))
                h2p = qp.tile([P, G2], f8, tag="h2p", bufs=3)
                nc.scalar.activation(h2p[:, :], psh2[:, :G2], AF.Copy)
                kb = b // BPC
                nc.scalar.dma_start(
                    out=h2bs[kb][(b % BPC) * P:(b % BPC + 1) * P, :],
                    in_=h2p[:, :])
                if b % BPC == BPC - 1:
                    agt2 = (h2gA[kb * CHR:(kb + 1) * CHR, :]
                            if kb < SPL2 else
                            h2gB[(kb - SPL2) * CHR:(kb - SPL2 + 1) * CHR, :])
                    nc.gpsimd.collective_compute(
                        "AllGather", OP.bypass,
                        replica_groups=[list(range(cfg.NC))],
                        ins=[h2bs[kb][:, :].opt()],
                        outs=[agt2.opt()])

            def l1_round(meta, sb_base, table, mode):
                # mode: 0 = spill-init, 1 = spill-add, 2 = finalize
                for q in range(meta["nblocks"] // 32):
                    r0 = (sb_base + q * 4) * P
                    gt = sp.tile([P, 32 * G1], f8, tag="gt1", bufs=3)
                    ix = sp.tile([P, 32], dt.int32, tag="ix1")
                    nc.sync.dma_start(
                        out=ix[:, :].rearrange("p (a e) -> p a e", e=8),
                        in_=idx1_d[r0:r0 + 4 * P, :]
                            .rearrange("(a p) e -> p a e", p=P))
                    nc.gpsimd.indirect_dma_start(
                        out=gt[:, :], out_offset=None, in_=table[:, :],
                        in_offset=bass.IndirectOffsetOnAxis(ap=ix[:, :],
                                                            axis=0))
                    Ssb = sp.tile([P, 32 * P], f8, tag="S1", bufs=2)
                    nc.sync.dma_start(
                        out=Ssb[:, :].rearrange("p (a v) -> p a v", v=8 * P),
                        in_=Sm1_d[r0:r0 + 4 * P, :]
                            .rearrange("(a p) v -> p a v", p=P))
                    for j in range(32):
                        g = q * 32 + j
                        b = int(meta["b_of"][g])
                        first = bool(meta["first"][g])
                        last = bool(meta["last"][g])
                        if first:
                            psb = psS.tile([P, G1], f32, tag="agg",
                                           name="agg1")
                            ps_by_b[b] = psb
                        psb = ps_by_b[b]
                        stop = last and (not sched["g1b_nz"]
                                         if mode == 2 else True)
                        nc.tensor.matmul(
                            psb[:, :], lhsT=Ssb[:, j * P:(j + 1) * P],
                            rhs=gt[:, j * G1:(j + 1) * G1],
                            start=first, stop=stop)
                        if not last:
                            continue
                        if mode == 0:
                            nc.vector.tensor_copy(
                                aggA_s[:, b * G1:(b + 1) * G1],
                                ps_by_b.pop(b)[:, :])
                        elif mode == 1:
                            nc.vector.tensor_add(
                                aggA_s[:, b * G1:(b + 1) * G1],
                                ps_by_b.pop(b)[:, :],
                                aggA_s[:, b * G1:(b + 1) * G1])
                        else:
                            l1_finalize(b)

            sb = 0
            for r in range(3):
                l1_round(L1R[r], sb, h1gs[r], r)
                sb += L1R[r]["nblocks"] // 8

            # ================= LAYER 2 scatter (2 rounds) =================
            ps2 = {}

            def l2_finalize(b):
                psb2 = ps2.pop(b)
                if sched["g2b_nz"]:
                    nc.tensor.matmul(
                        psb2[:, :G2], lhsT=sdeg_s[:1, b * P:(b + 1) * P],
                        rhs=g2b_s[:1, :], start=False, stop=True,
                        skip_group_check=True)
                g2pre = qp.tile([P, G2], bf, tag="g2pre")
                nc.vector.tensor_add(g2pre[:, :], psb2[:, :G2],
                                     agg2_s[:, b * G2:(b + 1) * G2])
                g2t = qp.tile([P, G2], bf, tag="g2t")
                nc.scalar.activation(g2t[:, :], g2pre[:, :], AF.Relu,
                                     scale=dl2_s[:, b:b + 1])
                pstg = psT.tile([P, P], bf, tag="t")
                nc.tensor.transpose(pstg[:G2, :], g2t[:, :], ident[:, :])
                g2T = qp.tile([G2, P], bf, tag="g2T")
                nc.vector.tensor_copy(g2T[:, :], pstg[:G2, :])
                psf = psB.tile([P, 512], f32, tag="b")
                nc.tensor.matmul(psf[:, :FOUT], lhsT=g2T[:, :],
                                 rhs=fcw_s[:, :], start=True,
                                 stop=not sched["fcb_nz"])
                if sched["fcb_nz"]:
                    nc.tensor.matmul(psf[:, :FOUT], lhsT=ones1[:1, :],
                                     rhs=fcb_s[:1, :], start=False,
                                     stop=True, skip_group_check=True)
                nc.vector.tensor_copy(
                    out_acc[:, b * FOUT:(b + 1) * FOUT], psf[:, :FOUT])

            def l2_round(meta, sb_base, table, is_b):
                for q in range(meta["nblocks"] // 32):
                    r0 = (sb_base + q * 4) * P
                    gt2 = sp.tile([P, 32 * G2], f8, tag="gt2", bufs=2)
                    ix2 = sp.tile([P, 32], dt.int32, tag="ix2")
                    nc.sync.dma_start(
                        out=ix2[:, :].rearrange("p (a e) -> p a e", e=8),
                        in_=idx2_d[r0:r0 + 4 * P, :]
                            .rearrange("(a p) e -> p a e", p=P))
                    nc.gpsimd.indirect_dma_start(
                        out=gt2[:, :], out_offset=None, in_=table[:, :],
                        in_offset=bass.IndirectOffsetOnAxis(ap=ix2[:, :],
                                                            axis=0))
                    S2 = sp.tile([P, 32 * P], f8, tag="S2", bufs=2)
                    nc.sync.dma_start(
                        out=S2[:, :].rearrange("p (a v) -> p a v", v=8 * P),
                        in_=Sm2_d[r0:r0 + 4 * P, :]
                            .rearrange("(a p) v -> p a v", p=P))
                    for j in range(32):
                        g = q * 32 + j
                        b = int(meta["b_of"][g])
                        first = bool(meta["first"][g])
                        last = bool(meta["last"][g])
                        if first:
                            psb2 = psS.tile([P, G1], f32, tag="agg",
                                            name="agg2")
                            ps2[b] = psb2
                        psb2 = ps2[b]
                        stop = last and (not sched["g2b_nz"] if is_b else True)
                        nc.tensor.matmul(
                            psb2[:, :G2], lhsT=S2[:, j * P:(j + 1) * P],
                            rhs=gt2[:, j * G2:(j + 1) * G2],
                            start=first, stop=stop)
                        if not last:
                            continue
                        if not is_b:
                            nc.vector.tensor_copy(
                                agg2_s[:, b * G2:(b + 1) * G2],
                                ps2.pop(b)[:, :G2])
                        else:
                            l2_finalize(b)

            l2_round(L2A, 0, h2gA, False)
            l2_round(L2B, L2A["nblocks"] // 8, h2gB, True)

            # batched log_softmax over all node blocks (logits are tiny:
            # exp without max-shift is safe)
            e_all = qp.tile([P, NB * FOUT], f32, tag="eall", bufs=1)
            nc.scalar.activation(e_all[:, :], out_acc[:, :], AF.Exp)
            sums = qp.tile([P, NB], f32, tag="sums", bufs=1)
            nc.vector.reduce_sum(
                sums[:, :],
                e_all[:, :].rearrange("p (b f) -> p b f", f=FOUT),
                axis=AX.X)
            lns = qp.tile([P, NB], f32, tag="lns", bufs=1)
            nc.scalar.activation(lns[:, :], sums[:, :], AF.Ln)
            res = qp.tile([P, NB * FOUT], f32, tag="eall", bufs=1, name="res")
            nc.vector.tensor_tensor(
                out=res[:, :].rearrange("p (b f) -> p b f", f=FOUT),
                in0=out_acc[:, :].rearrange("p (b f) -> p b f", f=FOUT),
                in1=lns[:, :].unsqueeze(2).to_broadcast([P, NB, FOUT]),
                op=OP.subtract)
            nc.scalar.dma_start(
                out=out_d[:, :].rearrange("(b p) f -> p b f", p=P),
                in_=res[:, :].rearrange("p (b f) -> p b f", f=FOUT))
            if DBG:
                # sample blocks across h1g0/h1bR0 to identify AG layout
                probes = ([(h1gs[0], r) for r in
                           [0, 896, 1792, 2688, 3584, 7168, 10752, 14208]] +
                          [(h1bs[0], r) for r in [0, 448, 768]])
                for i, (tsrc, r0) in enumerate(probes):
                    tt = qp.tile([P, G1], f8, tag="dbgt", bufs=2)
                    nc.sync.dma_start(out=tt[:, :], in_=tsrc[r0:r0 + P, :])
                    nc.sync.dma_start(out=dbg_t[i * P:(i + 1) * P, :],
                                      in_=tt[:, :])
    return nc


_LAST_EXEC_NS = None
_LAST_RESULT = None


def run(inputs, cfg, trace=False, debug=False):
    global _LAST_EXEC_NS, _LAST_RESULT
    in_maps, sched = host_prep(inputs, cfg)
    nc = build(cfg, sched, debug=debug)
    nc.finalize()
    from concourse import bass_utils
    res = bass_utils.run_bass_kernel_spmd(
        nc, in_maps, core_ids=list(range(cfg.NC)), trace=trace)
    _LAST_EXEC_NS = res.exec_time_ns
    _LAST_RESULT = res
    outs = [np.asarray(res.results[c]["out"])[:cfg.NLOC_RAW]
            for c in range(cfg.NC)]
    return np.concatenate(outs, 0).astype(np.float32)


def kernel(**inputs):
    return run(inputs, _Cfg(**CFG_FULL))


# revision 48
# speedup vs baseline: 1.0095x; 1.0095x over previous
"""Distributed Trainium2 kernel for the GNN message-passing model.

Self-contained: host-side structural prep (sharding, edge sort, index
remap) + Bass/Tile SPMD kernel across 8 NeuronCores.

Math (see reference):
  logits = MLP(x1); m = 0.15 + 0.55*onehot(argmax(logits))
  r1 = (m@W1.sum(-1))*x2 + m@bp1
  g1 = relu(Dh A Dh (r1@gcn1_w) + gcn1_b); g1 = (m@W12)*g1 + 2e-4*(r1@W13)
  r2 = (m@W2.sum(-1))*g1 + m@bp2
  g2 = relu(Dh A Dh (r2@gcn2_w) + gcn2_b)
  out = log_softmax(g2@fc_w + fc_b)
where Dh = diag(deg^-1/2), deg = in-degree over dst.

Distribution: nodes sharded contiguously over 8 cores. Per GCN layer the
scaled features h' = Dh*h are AllGathered in fp8 (in node chunks, so comm
overlaps the producer pipeline); each core gathers h'[src] for edges
whose dst it owns via indirect DMA and scatter-reduces them with
one-hot matmuls on the TensorEngine (PSUM accumulation per dst block).
The router MLP and the r1->(gcn1_w|W13) matmuls run in fp8 DoubleRow
(2 fp8 MACs/cell/cycle); scatter masks/tables and collective payloads
are fp8e4m3 with power-of-two scaling. Layer-1 scatter runs in three
rounds keyed to AllGather chunk arrival so scatter work overlaps the
front; partial aggregates accumulate in SBUF (bf16).
"""

import numpy as np

P = 128
TAU_HI = 0.7
TAU_LO = 0.15   # (1-0.7)/2
S1 = 512.0      # fp8 scale for layer-1 h' table
S2 = 32768.0    # fp8 scale for layer-2 h' table
SZ = 524288.0   # fp8 scale for the z (W13) term
SW = 1024.0     # fp8 scale for mlp_w1/mlp_w2/gcn1_w
SW3 = 2048.0    # fp8 scale for mlp_w3
SH = 32.0       # fp8 scale for MLP hidden activations + r1
SWZ = float(2 ** 22)  # fp8 scale for W13*2e-4 inside the fused rhs


class _Cfg:
    def __init__(self, N, E, F1=768, H=512, G1=256, G2=32, FOUT=40, C=7):
        self.NC = 8
        self.N = N
        self.E = E
        self.NLOC_RAW = N // self.NC
        self.NB = -(-self.NLOC_RAW // P)          # node blocks per core
        self.NLOC = self.NB * P
        assert self.NB % C == 0, (self.NB, C)
        self.C = C                                 # allgather chunks
        self.BPC = self.NB // C                    # blocks per chunk
        self.CH = self.BPC * P                     # chunk nodes
        self.TR = self.NC * self.NLOC              # gathered table rows
        self.CHR = self.NC * self.CH               # rows per chunk in table
        self.F1, self.H, self.G1, self.G2, self.FOUT = F1, H, G1, G2, FOUT
        self.KF1 = F1 // P                         # 6 k-tiles
        self.KH = H // P                           # 4
        self.KG1 = G1 // P                         # 2
        self.SPL1 = [1, 4]                         # L1 round chunk splits
        self.SPL2 = 3                  # L2 round-A src chunks
        self.NFREE = min(448, self.CH)             # front free-dim unit
        assert self.CH % self.NFREE == 0
        self.FU = self.CH // self.NFREE            # free units per chunk


CFG_FULL = dict(N=50000, E=800000)


def _to_bf16(x):
    import ml_dtypes
    return np.asarray(x, np.float32).astype(ml_dtypes.bfloat16)


def _to_f8(x, scale=1.0):
    import ml_dtypes
    return np.clip(np.asarray(x, np.float32) * scale, -240, 240).astype(
        ml_dtypes.float8_e4m3)


def _rows_l1(v, cfg):
    """Gathered-table row for global node id v (vectorized).

    One AllGather per chunk k, each writing the [k*CHR, (k+1)*CHR) slice
    of the (conceptual) global table as a rank-major concat of that
    chunk: row = k*CHR + c*CH + off. Round tables are contiguous slices
    of this global row space (build_layout subtracts the round base)."""
    c = v // cfg.NLOC_RAW
    s = v - c * cfg.NLOC_RAW
    k = s // cfg.CH
    return k * cfg.CHR + c * cfg.CH + (s - k * cfg.CH)


_rows_l2 = _rows_l1


def host_prep(inputs, cfg):
    """Returns (in_maps, sched). sched is baked into the built graph and
    must be identical for every core (SPMD)."""
    x1 = np.asarray(inputs["x1"], np.float32)
    x2 = np.asarray(inputs["x2"], np.float32)
    ei = np.asarray(inputs["edge_index"])
    src = ei[0].astype(np.int64)
    dst = ei[1].astype(np.int64)
    N, E, NC = cfg.N, cfg.E, cfg.NC
    assert x1.shape[0] == N and src.shape[0] == E

    deg = np.bincount(dst, minlength=N).astype(np.float64)
    dinv = np.where(deg > 0, deg ** -0.5, 0.0).astype(np.float32)
    sdeg = np.sqrt(deg).astype(np.float32)  # 1/dinv where deg>0 else 0

    # ---- per-core edge partition by dst owner, sorted by dst block ----
    owner = dst // cfg.NLOC_RAW
    dloc = dst - owner * cfg.NLOC_RAW
    dblk = dloc // P
    drel_all = (dloc - dblk * P).astype(np.float32)
    rows_l1 = _rows_l1(src, cfg).astype(np.int32)
    rows_l2 = _rows_l2(src, cfg).astype(np.int32)

    def split_core(rows_all):
        per_core = []
        for c in range(NC):
            sel = np.where(owner == c)[0]
            order = np.argsort(dblk[sel], kind="stable")
            sel = sel[order]
            b_of = dblk[sel]
            bounds = np.searchsorted(b_of, np.arange(cfg.NB + 1))
            lists = []
            for b in range(cfg.NB):
                idxs = sel[bounds[b]:bounds[b + 1]]
                lists.append((rows_all[idxs], drel_all[idxs]))
            per_core.append(lists)
        return per_core

    per_core_l1 = split_core(rows_l1)
    per_core_l2 = split_core(rows_l2)

    # Uniform cross-core layouts split in rounds by src chunk
    # (round boundary = which AllGather chunks the gathers depend on).
    def build_layout(per_core, chunk_splits, pad_mult):
        bounds_k = [0] + [s * cfg.CHR for s in chunk_splits] + [cfg.C * cfg.CHR]
        layout = dict(rounds=[])
        for r in range(len(bounds_k) - 1):
            lo, hi = bounds_k[r], bounds_k[r + 1]
            cntr = np.zeros((NC, cfg.NB), np.int64)
            per_rc = []
            for c in range(NC):
                pc = []
                for b in range(cfg.NB):
                    rows, rel = per_core[c][b]
                    m = (rows >= lo) & (rows < hi)
                    pc.append((rows[m] - lo, rel[m]))
                    cntr[c, b] = int(m.sum())
                per_rc.append(pc)
            Kb = np.maximum(1, -(-cntr.max(axis=0) // P)).astype(np.int64)
            nb_round = int(Kb.sum())
            pad = (-nb_round) % pad_mult
            nb_round += pad
            b_of = np.concatenate([np.repeat(np.arange(cfg.NB), Kb),
                                   np.full(pad, cfg.NB - 1)])
            first = np.zeros(nb_round, bool)
            last = np.zeros(nb_round, bool)
            off = 0
            for b in range(cfg.NB):
                first[off] = True
                e = off + int(Kb[b])
                if b == cfg.NB - 1:
                    e = nb_round
                last[e - 1] = True
                off += int(Kb[b])
            layout["rounds"].append(dict(Kb=Kb, nblocks=nb_round, b_of=b_of,
                                         first=first, last=last,
                                         per_rc=per_rc))
        return layout

    lay1 = build_layout(per_core_l1, cfg.SPL1, 32)
    lay2 = build_layout(per_core_l2, [cfg.SPL2], 32)

    def pack_layout(layout, c):
        idxs, Ss = [], []
        for rr in layout["rounds"]:
            sbs = rr["nblocks"] // 8
            idx = np.zeros((sbs * P, 8), np.int32)
            drl = np.full((sbs * P, 8), -1.0, np.float32)
            g = 0
            for b in range(cfg.NB):
                rows, rel = rr["per_rc"][c][b]
                n = len(rows)
                nblk = int(rr["Kb"][b])
                if b == cfg.NB - 1:
                    nblk = rr["nblocks"] - g
                for j in range(nblk):
                    s, jj = g // 8, g % 8
                    e0 = j * P
                    m = min(P, max(0, n - e0))
                    if m > 0:
                        idx[s * P:s * P + m, jj] = rows[e0:e0 + m]
                        drl[s * P:s * P + m, jj] = rel[e0:e0 + m]
                    g += 1
            idxs.append(idx)
            Ss.append(_to_f8(
                (drl[:, :, None] ==
                 np.arange(P, dtype=np.float32)[None, None, :])
                .astype(np.float32).reshape(sbs * P, 8 * P)))
        return (np.concatenate(idxs, axis=0), np.concatenate(Ss, axis=0))

    sched = dict(lay1=[dict(nblocks=r["nblocks"], b_of=r["b_of"],
                            first=r["first"], last=r["last"])
                       for r in lay1["rounds"]],
                 lay2=[dict(nblocks=r["nblocks"], b_of=r["b_of"],
                            first=r["first"], last=r["last"])
                       for r in lay2["rounds"]])

    # ---- weights ----
    w1 = np.asarray(inputs["mlp_w1"], np.float32)
    w2 = np.asarray(inputs["mlp_w2"], np.float32)
    w3 = np.asarray(inputs["mlp_w3"], np.float32)
    b1 = np.asarray(inputs["mlp_b1"], np.float32)
    b2 = np.asarray(inputs["mlp_b2"], np.float32)
    b3 = np.asarray(inputs["mlp_b3"], np.float32)
    W1s = np.asarray(inputs["W1"], np.float32).sum(-1)
    W12 = np.asarray(inputs["W12"], np.float32)
    W13 = np.asarray(inputs["W13"], np.float32) * 2e-4
    bp1 = np.asarray(inputs["bp1"], np.float32)
    W2s = np.asarray(inputs["W2"], np.float32).sum(-1)
    bp2 = np.asarray(inputs["bp2"], np.float32)
    g1w = np.asarray(inputs["gcn1_w"], np.float32)
    g1b = np.asarray(inputs["gcn1_b"], np.float32)
    g2w = np.asarray(inputs["gcn2_w"], np.float32)
    g2b = np.asarray(inputs["gcn2_b"], np.float32)
    fcw = np.asarray(inputs["fc_w"], np.float32)
    fcb = np.asarray(inputs["fc_b"], np.float32)

    sched["bp1_nz"] = bool(np.any(bp1 != 0))
    sched["bp2_nz"] = bool(np.any(bp2 != 0))
    sched["g1b_nz"] = bool(np.any(g1b != 0))
    sched["g2b_nz"] = bool(np.any(g2b != 0))
    sched["fcb_nz"] = bool(np.any(fcb != 0))
    sched["b3_nz"] = bool(np.any(b3 != 0))

    def pack_lhsT(w, KT, MT, conv):
        o = np.zeros((P, KT * MT * P), np.float32)
        for k in range(KT):
            for m in range(MT):
                o[:, (k * MT + m) * P:(k * MT + m + 1) * P] = \
                    w[k * P:(k + 1) * P, m * P:(m + 1) * P]
        return conv(o)

    def pack_rhs(w, KT, F, conv):
        o = np.zeros((P, KT * F), np.float32)
        for k in range(KT):
            o[:, k * F:(k + 1) * F] = w[k * P:(k + 1) * P, :]
        return conv(o)

    def pack_k3(w, F):
        o = np.zeros((4, F), np.float32)
        o[:3] = w
        return _to_bf16(o)

    w1_p = pack_lhsT(w1 * SW, cfg.KF1, cfg.KH, _to_f8)
    w2_p = pack_lhsT(w2 * SW, cfg.KH, cfg.KH, _to_f8)
    w3_p = pack_rhs(np.pad(w3, ((0, 0), (0, 1))) * SW3, cfg.KH, 4, _to_f8)
    b1_p = b1.reshape(cfg.KH, P).T.copy() * SH  # ACT bias adds after scale
    b2_p = b2.reshape(cfg.KH, P).T.copy() * SH
    b3_p = (np.pad(b3, (0, 1)) * (SH * SW3)).reshape(1, 4).repeat(P, 0).copy()
    # fused [g1w*SW | W13*2e-4*SWZ] rhs: per k-tile 512 wide, fp8
    g1f = np.concatenate([g1w * SW, W13 * SWZ], axis=1)  # [768, 512]
    g1f_p = pack_rhs(g1f, cfg.KF1, 2 * cfg.G1, _to_f8)
    g2w_p = pack_rhs(g2w, cfg.KG1, cfg.G2, _to_bf16)
    fcw_p = _to_bf16(fcw)
    W1s_p = pack_k3(W1s, cfg.F1)
    bp1_p = pack_k3(bp1 * SH, cfg.F1)
    W12_p = pack_k3(W12, cfg.G1)
    W2s_p = pack_k3(W2s, cfg.G1)
    bp2_p = pack_k3(bp2, cfg.G1)
    g1b_p = _to_bf16(g1b.reshape(1, cfg.G1) * S1)
    g2b_p = _to_bf16(g2b.reshape(1, cfg.G2) * S2)
    fcb_p = _to_bf16(fcb.reshape(1, cfg.FOUT))

    in_maps = []
    for c in range(NC):
        lo = c * cfg.NLOC_RAW
        hi = lo + cfg.NLOC_RAW
        x1T = np.zeros((cfg.F1, cfg.NLOC), np.float32)
        x1T[:, :cfg.NLOC_RAW] = x1[lo:hi].T
        x2T = np.zeros((cfg.F1, cfg.NLOC), np.float32)
        x2T[:, :cfg.NLOC_RAW] = x2[lo:hi].T
        dinv_t = np.zeros((P, cfg.NB), np.float32)
        dinv_t.T.reshape(-1)[:cfg.NLOC_RAW] = dinv[lo:hi]
        sdeg_r = np.zeros((1, cfg.NLOC), np.float32)
        sdeg_r[0, :cfg.NLOC_RAW] = sdeg[lo:hi]

        ident_np = _to_bf16(np.eye(P, dtype=np.float32))
        idx1, Sm1 = pack_layout(lay1, c)
        idx2, Sm2 = pack_layout(lay2, c)
        im = {
            "ident": ident_np,
            "x1T": _to_f8(x1T), "x2T": _to_f8(x2T),
            "idx1": idx1, "Sm1": Sm1, "idx2": idx2, "Sm2": Sm2,
            "dh1": dinv_t * (S1 / (SH * SW)), "dl1": dinv_t * (1.0 / S1),
            "dh2": dinv_t * S2, "dl2": dinv_t * (1.0 / S2),
            "sdeg_r": _to_bf16(sdeg_r),
            "w1": w1_p, "w2": w2_p, "w3": w3_p,
            "b1": b1_p, "b2": b2_p, "b3": b3_p,
            "g1f": g1f_p, "g2w": g2w_p, "fcw": fcw_p,
            "W1s": W1s_p, "bp1": bp1_p, "W12": W12_p, "W2s": W2s_p,
            "bp2": bp2_p, "g1b": g1b_p, "g2b": g2b_p, "fcb": fcb_p,
        }
        in_maps.append(im)
    return in_maps, sched


def build(cfg, sched, debug=False):
    import concourse.bacc as bacc
    import concourse.bass as bass
    import concourse.mybir as mybir
    import concourse.tile as tile

    dt = mybir.dt
    AF = mybir.ActivationFunctionType
    OP = mybir.AluOpType
    AX = mybir.AxisListType
    DR = mybir.MatmulPerfMode.DoubleRow

    nc = bacc.Bacc("TRN2", target_bir_lowering=False, debug=debug)

    NB, C, BPC, CH, NLOC, TR, CHR = (cfg.NB, cfg.C, cfg.BPC, cfg.CH,
                                     cfg.NLOC, cfg.TR, cfg.CHR)
    F1, H, G1, G2, FOUT = cfg.F1, cfg.H, cfg.G1, cfg.G2, cfg.FOUT
    KF1, KH, KG1 = cfg.KF1, cfg.KH, cfg.KG1
    NF, FU = cfg.NFREE, cfg.FU
    L1R = sched["lay1"]            # 3 rounds
    L2A, L2B = sched["lay2"]
    SB1 = sum(r["nblocks"] for r in L1R) // 8
    SB2T = (L2A["nblocks"] + L2B["nblocks"]) // 8
    SPL1, SPL2 = cfg.SPL1, cfg.SPL2

    bf = dt.bfloat16
    f32 = dt.float32
    f8 = dt.float8e4

    dd = {}

    def din(name, shape, dtype):
        dd[name] = nc.declare_dram_parameter(name, list(shape), dtype,
                                             isOutput=False)
        return dd[name]

    x1T_d = din("x1T", [F1, NLOC], f8)
    x2T_d = din("x2T", [F1, NLOC], f8)
    idx1_d = din("idx1", [SB1 * P, 8], dt.int32)
    Sm1_d = din("Sm1", [SB1 * P, 8 * P], f8)
    idx2_d = din("idx2", [SB2T * P, 8], dt.int32)
    Sm2_d = din("Sm2", [SB2T * P, 8 * P], f8)
    dh1_d = din("dh1", [P, NB], f32)
    dl1_d = din("dl1", [P, NB], f32)
    dh2_d = din("dh2", [P, NB], f32)
    dl2_d = din("dl2", [P, NB], f32)
    sdeg_d = din("sdeg_r", [1, NLOC], bf)
    w1_d = din("w1", [P, KF1 * KH * P], f8)
    w2_d = din("w2", [P, KH * KH * P], f8)
    w3_d = din("w3", [P, KH * 4], f8)
    b1_d = din("b1", [P, KH], f32)
    b2_d = din("b2", [P, KH], f32)
    b3_d = din("b3", [P, 4], f32)
    g1f_d = din("g1f", [P, KF1 * 2 * G1], f8)
    g2w_d = din("g2w", [P, KG1 * G2], bf)
    fcw_d = din("fcw", [G2, FOUT], bf)
    W1s_d = din("W1s", [4, F1], bf)
    bp1_d = din("bp1", [4, F1], bf)
    W12_d = din("W12", [4, G1], bf)
    W2s_d = din("W2s", [4, G1], bf)
    bp2_d = din("bp2", [4, G1], bf)
    g1b_d = din("g1b", [1, G1], bf)
    g2b_d = din("g2b", [1, G2], bf)
    fcb_d = din("fcb", [1, FOUT], bf)
    ident_d = din("ident", [P, P], bf)
    out_d = nc.declare_dram_parameter("out", [NLOC, FOUT], f32, isOutput=True)
    import os
    DBG = bool(os.environ.get("K_DBG"))
    if DBG:
        dbg_n = 11
        dbg_t = nc.declare_dram_parameter("dbg_t", [dbg_n * P, G1], f8,
                                          isOutput=True)

    with tile.TileContext(nc) as tc:
        with (
            tc.tile_pool(name="const", bufs=1) as cp,
            tc.tile_pool(name="front", bufs=2) as fp,
            tc.tile_pool(name="scat", bufs=3) as sp,
            tc.tile_pool(name="fin", bufs=2) as qp,
            tc.tile_pool(name="psG", bufs=2, space="PSUM") as psG,
            tc.tile_pool(name="psH", bufs=2, space="PSUM") as psH,
            tc.tile_pool(name="psS", bufs=2, space="PSUM") as psS,
            tc.tile_pool(name="psB", bufs=1, space="PSUM") as psB,
            tc.tile_pool(name="psT", bufs=1, space="PSUM") as psT,
            tc.tile_pool(name="dram", bufs=1, space="DRAM") as dp,
        ):
            def load(dr, shape, dtype, name):
                t = cp.tile(shape, dtype, tag=name)
                nc.sync.dma_start(out=t[:, :], in_=dr[:, :])
                return t

            w1_s = load(w1_d, [P, KF1 * KH * P], f8, "w1")
            w2_s = load(w2_d, [P, KH * KH * P], f8, "w2")
            w3_s = load(w3_d, [P, KH * 4], f8, "w3")
            b1_s = load(b1_d, [P, KH], f32, "b1")
            b2_s = load(b2_d, [P, KH], f32, "b2")
            b3_s = load(b3_d, [P, 4], f32, "b3")
            g1f_s = load(g1f_d, [P, KF1 * 2 * G1], f8, "g1f")
            g2w_s = load(g2w_d, [P, KG1 * G2], bf, "g2w")
            fcw_s = load(fcw_d, [G2, FOUT], bf, "fcw")
            W1s_s = load(W1s_d, [4, F1], bf, "W1s")
            bp1_s = load(bp1_d, [4, F1], bf, "bp1")
            W12_s = load(W12_d, [4, G1], bf, "W12")
            W2s_s = load(W2s_d, [4, G1], bf, "W2s")
            bp2_s = load(bp2_d, [4, G1], bf, "bp2")
            g1b_s = load(g1b_d, [1, G1], bf, "g1b")
            g2b_s = load(g2b_d, [1, G2], bf, "g2b")
            fcb_s = load(fcb_d, [1, FOUT], bf, "fcb")
            dh1_s = load(dh1_d, [P, NB], f32, "dh1")
            dl1_s = load(dl1_d, [P, NB], f32, "dl1")
            dh2_s = load(dh2_d, [P, NB], f32, "dh2")
            dl2_s = load(dl2_d, [P, NB], f32, "dl2")
            sdeg_s = load(sdeg_d, [1, NLOC], bf, "sdeg")

            ident = load(ident_d, [P, P], bf, "ident")
            ones1 = cp.tile([1, P], bf, tag="ones1")
            nc.vector.memset(ones1[:, :], 1.0)

            mT_s = cp.tile([4, NLOC], bf, tag="mT")
            out_acc = cp.tile([P, NB * FOUT], f32, tag="oacc")
            z_s = cp.tile([P, NB * G1], f8, tag="z")
            aggA_s = cp.tile([P, NB * G1], bf, tag="aggA")
            agg2_s = cp.tile([P, NB * G2], bf, tag="agg2")

            n_r1 = [SPL1[0], SPL1[1] - SPL1[0], C - SPL1[1]]
            h1bs = [dp.tile([CH, G1], f8, tag=f"h1b{k}", name=f"h1b{k}")
                    for k in range(C)]
            h2bs = [dp.tile([CH, G2], f8, tag=f"h2b{k}", name=f"h2b{k}")
                    for k in range(C)]
            h1gs = [dp.tile([n_r1[r] * CHR, G1], f8, tag=f"h1g{r}",
                            name=f"h1g{r}")
                    for r in range(3)]
            h2gA = dp.tile([SPL2 * CHR, G2], f8, tag="h2gA")
            h2gB = dp.tile([(C - SPL2) * CHR, G2], f8, tag="h2gB")

            # ================= FRONT (per chunk) =================
            for k in range(C):
                n0 = k * CH
                x1c = fp.tile([P, KF1 * CH], f8, tag="x1c")
                nc.sync.dma_start(
                    out=x1c[:, :].rearrange("p (a n) -> p a n", n=CH),
                    in_=x1T_d[:, n0:n0 + CH].rearrange("(a p) n -> p a n", p=P))
                x2c = fp.tile([P, KF1 * CH], f8, tag="x2c", bufs=2)
                nc.sync.dma_start(
                    out=x2c[:, :].rearrange("p (a n) -> p a n", n=CH),
                    in_=x2T_d[:, n0:n0 + CH].rearrange("(a p) n -> p a n", p=P))

                x1c3 = x1c[:, :].rearrange("p (a n) -> p a n", n=CH)
                w1c4 = w1_s[:, :].rearrange("p (a m c) -> p a m c", m=KH, c=P)
                h1T = fp.tile([P, KH * CH], f8, tag="h1T", bufs=1)
                for u in range(FU):
                    for m in range(KH):
                        ps = psG.tile([P, NF], f32, tag="g")
                        for kk in range(0, KF1, 2):
                            nc.tensor.matmul(
                                ps[:, :],
                                lhsT=w1c4[:, kk:kk + 2, m:m + 1, :],
                                rhs=x1c3[:, kk:kk + 2,
                                         u * NF:u * NF + NF],
                                perf_mode=DR,
                                start=(kk == 0), stop=(kk == KF1 - 2))
                        nc.scalar.activation(
                            h1T[:, m * CH + u * NF:m * CH + u * NF + NF],
                            ps[:, :], AF.Relu, bias=b1_s[:, m:m + 1],
                            scale=SH / SW)
                h1T3 = h1T[:, :].rearrange("p (a n) -> p a n", n=CH)
                w2c4 = w2_s[:, :].rearrange("p (a m c) -> p a m c", m=KH, c=P)
                h2T = fp.tile([P, KH * CH], f8, tag="h2T", bufs=1)
                for u in range(FU):
                    for m in range(KH):
                        ps = psG.tile([P, NF], f32, tag="g")
                        for kk in range(0, KH, 2):
                            nc.tensor.matmul(
                                ps[:, :],
                                lhsT=w2c4[:, kk:kk + 2, m:m + 1, :],
                                rhs=h1T3[:, kk:kk + 2,
                                         u * NF:u * NF + NF],
                                perf_mode=DR,
                                start=(kk == 0), stop=(kk == KH - 2))
                        nc.scalar.activation(
                            h2T[:, m * CH + u * NF:m * CH + u * NF + NF],
                            ps[:, :], AF.Relu, bias=b2_s[:, m:m + 1],
                            scale=1.0 / SW)

                mmc = fp.tile([P, BPC * 3], bf, tag="mmc")
                for nb in range(BPC):
                    psl = psB.tile([P, 512], f32, tag="b")
                    for kk in range(KH):
                        nc.tensor.matmul(
                            psl[:, :4],
                            lhsT=h2T[:, kk * CH + nb * P:kk * CH + (nb + 1) * P],
                            rhs=w3_s[:, kk * 4:(kk + 1) * 4],
                            start=(kk == 0), stop=(kk == KH - 1))
                    lg = fp.tile([P, 3], f32, tag="lg")
                    if sched["b3_nz"]:
                        nc.vector.tensor_add(lg[:, :], psl[:, :3], b3_s[:, :3])
                    else:
                        nc.vector.tensor_copy(lg[:, :], psl[:, :3])
                    rmax = fp.tile([P, 1], f32, tag="rmax")
                    nc.vector.reduce_max(rmax[:, :], lg[:, :], axis=AX.X)
                    mm = fp.tile([P, 3], bf, tag="mm")
                    nc.vector.tensor_scalar(
                        mm[:, :], lg[:, :], rmax[:, :1], None, OP.is_equal)
                    nc.scalar.activation(mmc[:, nb * 3:(nb + 1) * 3],
                                         mm[:, :], AF.Copy,
                                         bias=TAU_LO, scale=TAU_HI - TAU_LO)
                for nb in range(BPC):
                    b_glob = k * BPC + nb
                    pst = psT.tile([P, P], bf, tag="t")
                    nc.tensor.transpose(pst[:3, :],
                                        mmc[:, nb * 3:(nb + 1) * 3],
                                        ident[:, :])
                    nc.vector.tensor_copy(
                        mT_s[:3, b_glob * P:(b_glob + 1) * P], pst[:3, :])

                r1T = fp.tile([P, KF1 * CH], f8, tag="r1T")
                for u in range(FU):
                    for f in range(KF1):
                        psr = psG.tile([P, NF], f32, tag="g")
                        nc.tensor.matmul(
                            psr[:, :], lhsT=W1s_s[:3, f * P:(f + 1) * P],
                            rhs=mT_s[:3, n0 + u * NF:n0 + u * NF + NF],
                            start=True, stop=True)
                        if sched["bp1_nz"]:
                            psr2 = psB.tile([P, 512], f32, tag="b")
                            nc.tensor.matmul(
                                psr2[:, :NF], lhsT=bp1_s[:3, f * P:(f + 1) * P],
                                rhs=mT_s[:3, n0 + u * NF:n0 + u * NF + NF],
                                start=True, stop=True)
                            tmp = fp.tile([P, NF], f32, tag="r1tmp")
                            nc.vector.tensor_mul(
                                tmp[:, :], psr[:, :],
                                x2c[:, f * CH + u * NF:f * CH + u * NF + NF])
                            nc.vector.scalar_tensor_tensor(
                                out=r1T[:, f * CH + u * NF:f * CH + u * NF + NF],
                                in0=tmp[:, :], scalar=SH,
                                in1=psr2[:, :NF], op0=OP.mult, op1=OP.add)
                        else:
                            nc.vector.scalar_tensor_tensor(
                                out=r1T[:, f * CH + u * NF:f * CH + u * NF + NF],
                                in0=psr[:, :], scalar=SH,
                                in1=x2c[:, f * CH + u * NF:f * CH + u * NF + NF],
                                op0=OP.mult, op1=OP.mult)

                r1T3 = r1T[:, :].rearrange("p (a n) -> p a n", n=CH)
                g1f3 = g1f_s[:, :].rearrange("p (a c) -> p a c", c=2 * G1)
                ri = 0 if k < SPL1[0] else (1 if k < SPL1[1] else 2)
                k0 = [0, SPL1[0], SPL1[1]][ri]
                for nb in range(BPC):
                    b_glob = k * BPC + nb
                    psh = psH.tile([P, 2 * G1], f32, tag="h")
                    for f in range(0, KF1, 2):
                        nc.tensor.matmul(
                            psh[:, :],
                            lhsT=r1T3[:, f:f + 2, nb * P:(nb + 1) * P],
                            rhs=g1f3[:, f:f + 2, :],
                            perf_mode=DR,
                            start=(f == 0), stop=(f == KF1 - 2))
                    h1p = fp.tile([P, G1], f8, tag="h1p")
                    nc.scalar.activation(h1p[:, :], psh[:, :G1], AF.Copy,
                                         scale=dh1_s[:, b_glob:b_glob + 1])
                    nc.scalar.dma_start(
                        out=h1bs[k][nb * P:(nb + 1) * P, :], in_=h1p[:, :])
                    nc.scalar.activation(
                        z_s[:, b_glob * G1:(b_glob + 1) * G1],
                        psh[:, G1:2 * G1], AF.Copy, scale=SZ / (SH * SWZ))

                agt = h1gs[ri][(k - k0) * CHR:(k - k0 + 1) * CHR, :]
                nc.gpsimd.collective_compute(
                    "AllGather", OP.bypass,
                    replica_groups=[list(range(cfg.NC))],
                    ins=[h1bs[k][:, :].opt()],
                    outs=[agt.opt()])

            # ================= LAYER 1 scatter (3 rounds) =================
            ps_by_b = {}

            def l1_finalize(b):
                psb = ps_by_b.pop(b)
                if sched["g1b_nz"]:
                    nc.tensor.matmul(
                        psb[:, :], lhsT=sdeg_s[:1, b * P:(b + 1) * P],
                        rhs=g1b_s[:1, :], start=False, stop=True,
                        skip_group_check=True)
                g1pre = qp.tile([P, G1], bf, tag="g1pre", bufs=2)
                nc.vector.tensor_add(g1pre[:, :], psb[:, :],
                                     aggA_s[:, b * G1:(b + 1) * G1])
                g1r = qp.tile([P, G1], bf, tag="g1r", bufs=2)
                nc.scalar.activation(g1r[:, :], g1pre[:, :], AF.Relu,
                                     scale=dl1_s[:, b:b + 1])
                psmw = psH.tile([P, 2 * G1], f32, tag="h")
                nc.tensor.matmul(psmw[:, :G1],
                                 lhsT=mT_s[:3, b * P:(b + 1) * P],
                                 rhs=W12_s[:3, :], start=True, stop=True)
                nc.tensor.matmul(psmw[:, G1:2 * G1],
                                 lhsT=mT_s[:3, b * P:(b + 1) * P],
                                 rhs=W2s_s[:3, :], start=True, stop=True)
                g1t = qp.tile([P, G1], bf, tag="g1t", bufs=2)
                nc.vector.tensor_mul(g1t[:, :], g1r[:, :], psmw[:, :G1])
                g1v = qp.tile([P, G1], bf, tag="g1v", bufs=2)
                nc.vector.scalar_tensor_tensor(
                    out=g1v[:, :], in0=z_s[:, b * G1:(b + 1) * G1],
                    scalar=1.0 / SZ, in1=g1t[:, :],
                    op0=OP.mult, op1=OP.add)
                r2 = qp.tile([P, G1], bf, tag="r2", bufs=2)
                if sched["bp2_nz"]:
                    psm3 = psB.tile([P, 512], f32, tag="b")
                    nc.tensor.matmul(psm3[:, :G1],
                                     lhsT=mT_s[:3, b * P:(b + 1) * P],
                                     rhs=bp2_s[:3, :], start=True, stop=True)
                    r2u = qp.tile([P, G1], bf, tag="r2u")
                    nc.vector.tensor_mul(r2u[:, :], g1v[:, :],
                                         psmw[:, G1:2 * G1])
                    r2v = qp.tile([P, G1], bf, tag="r2v")
                    nc.vector.tensor_add(r2v[:, :], r2u[:, :], psm3[:, :G1])
                    nc.vector.tensor_scalar(r2[:, :], r2v[:, :],
                                            dh2_s[:, b:b + 1], None, OP.mult)
                else:
                    nc.vector.scalar_tensor_tensor(
                        out=r2[:, :], in0=g1v[:, :],
                        scalar=dh2_s[:, b:b + 1],
                        in1=psmw[:, G1:2 * G1], op0=OP.mult, op1=OP.mult)
                r2T = qp.tile([P, KG1 * P], bf, tag="r2T", bufs=2)
                for f in range(KG1):
                    pst = psT.tile([P, P], bf, tag="t")
                    nc.tensor.transpose(pst[:, :], r2[:, f * P:(f + 1) * P],
                                        ident[:, :])
                    nc.vector.tensor_copy(r2T[:, f * P:(f + 1) * P],
                                          pst[:, :])
                psh2 = psB.tile([P, 512], f32, tag="b")
                for f in range(KG1):
                    nc.tensor.matmul(
                        psh2[:, :G2], lhsT=r2T[:, f * P:(f + 1) * P],
                        rhs=g2w_s[:, f * G2:(f + 1) * G2],
                        start=(f == 0), stop=(f == KG1 - 1))
                h2p = qp.tile([P, G2], f8, tag="h2p", bufs=3)
                nc.scalar.activation(h2p[:, :], psh2[:, :G2], AF.Copy)
                kb = b // BPC
                nc.scalar.dma_start(
                    out=h2bs[kb][(b % BPC) * P:(b % BPC + 1) * P, :],
                    in_=h2p[:, :])
                if b % BPC == BPC - 1:
                    agt2 = (h2gA[kb * CHR:(kb + 1) * CHR, :]
                            if kb < SPL2 else
                            h2gB[(kb - SPL2) * CHR:(kb - SPL2 + 1) * CHR, :])
                    nc.gpsimd.collective_compute(
                        "AllGather", OP.bypass,
                        replica_groups=[list(range(cfg.NC))],
                        ins=[h2bs[kb][:, :].opt()],
                        outs=[agt2.opt()])

            def l1_round(meta, sb_base, table, mode):
                # mode: 0 = spill-init, 1 = spill-add, 2 = finalize
                for q in range(meta["nblocks"] // 32):
                    r0 = (sb_base + q * 4) * P
                    gt = sp.tile([P, 32 * G1], f8, tag="gt1", bufs=3)
                    ix = sp.tile([P, 32], dt.int32, tag="ix1")
                    nc.scalar.dma_start(
                        out=ix[:, :].rearrange("p (a e) -> p a e", e=8),
                        in_=idx1_d[r0:r0 + 4 * P, :]
                            .rearrange("(a p) e -> p a e", p=P))
                    nc.gpsimd.indirect_dma_start(
                        out=gt[:, :], out_offset=None, in_=table[:, :],
                        in_offset=bass.IndirectOffsetOnAxis(ap=ix[:, :],
                                                            axis=0))
                    Ssb = sp.tile([P, 32 * P], f8, tag="S1", bufs=2)
                    nc.sync.dma_start(
                        out=Ssb[:, :].rearrange("p (a v) -> p a v", v=8 * P),
                        in_=Sm1_d[r0:r0 + 4 * P, :]
                            .rearrange("(a p) v -> p a v", p=P))
                    for j in range(32):
                        g = q * 32 + j
                        b = int(meta["b_of"][g])
                        first = bool(meta["first"][g])
                        last = bool(meta["last"][g])
                        if first:
                            psb = psS.tile([P, G1], f32, tag="agg",
                                           name="agg1")
                            ps_by_b[b] = psb
                        psb = ps_by_b[b]
                        stop = last and (not sched["g1b_nz"]
                                         if mode == 2 else True)
                        nc.tensor.matmul(
                            psb[:, :], lhsT=Ssb[:, j * P:(j + 1) * P],
                            rhs=gt[:, j * G1:(j + 1) * G1],
                            start=first, stop=stop)
                        if not last:
                            continue
                        if mode == 0:
                            nc.scalar.activation(
                                aggA_s[:, b * G1:(b + 1) * G1],
                                ps_by_b.pop(b)[:, :], AF.Copy)
                        elif mode == 1:
                            nc.vector.tensor_add(
                                aggA_s[:, b * G1:(b + 1) * G1],
                                ps_by_b.pop(b)[:, :],
                                aggA_s[:, b * G1:(b + 1) * G1])
                        else:
                            l1_finalize(b)

            sb = 0
            for r in range(3):
                l1_round(L1R[r], sb, h1gs[r], r)
                sb += L1R[r]["nblocks"] // 8

            # ================= LAYER 2 scatter (2 rounds) =================
            ps2 = {}

            def l2_finalize(b):
                psb2 = ps2.pop(b)
                if sched["g2b_nz"]:
                    nc.tensor.matmul(
                        psb2[:, :G2], lhsT=sdeg_s[:1, b * P:(b + 1) * P],
                        rhs=g2b_s[:1, :], start=False, stop=True,
                        skip_group_check=True)
                g2pre = qp.tile([P, G2], bf, tag="g2pre")
                nc.vector.tensor_add(g2pre[:, :], psb2[:, :G2],
                                     agg2_s[:, b * G2:(b + 1) * G2])
                g2t = qp.tile([P, G2], bf, tag="g2t")
                nc.scalar.activation(g2t[:, :], g2pre[:, :], AF.Relu,
                                     scale=dl2_s[:, b:b + 1])
                pstg = psT.tile([P, P], bf, tag="t")
                nc.tensor.transpose(pstg[:G2, :], g2t[:, :], ident[:, :])
                g2T = qp.tile([G2, P], bf, tag="g2T")
                nc.vector.tensor_copy(g2T[:, :], pstg[:G2, :])
                psf = psB.tile([P, 512], f32, tag="b")
                nc.tensor.matmul(psf[:, :FOUT], lhsT=g2T[:, :],
                                 rhs=fcw_s[:, :], start=True,
                                 stop=not sched["fcb_nz"])
                if sched["fcb_nz"]:
                    nc.tensor.matmul(psf[:, :FOUT], lhsT=ones1[:1, :],
                                     rhs=fcb_s[:1, :], start=False,
                                     stop=True, skip_group_check=True)
                nc.vector.tensor_copy(
                    out_acc[:, b * FOUT:(b + 1) * FOUT], psf[:, :FOUT])

            def l2_round(meta, sb_base, table, is_b):
                for q in range(meta["nblocks"] // 32):
                    r0 = (sb_base + q * 4) * P
                    gt2 = sp.tile([P, 32 * G2], f8, tag="gt2", bufs=2)
                    ix2 = sp.tile([P, 32], dt.int32, tag="ix2")
                    nc.scalar.dma_start(
                        out=ix2[:, :].rearrange("p (a e) -> p a e", e=8),
                        in_=idx2_d[r0:r0 + 4 * P, :]
                            .rearrange("(a p) e -> p a e", p=P))
                    nc.gpsimd.indirect_dma_start(
                        out=gt2[:, :], out_offset=None, in_=table[:, :],
                        in_offset=bass.IndirectOffsetOnAxis(ap=ix2[:, :],
                                                            axis=0))
                    S2 = sp.tile([P, 32 * P], f8, tag="S2", bufs=2)
                    nc.sync.dma_start(
                        out=S2[:, :].rearrange("p (a v) -> p a v", v=8 * P),
                        in_=Sm2_d[r0:r0 + 4 * P, :]
                            .rearrange("(a p) v -> p a v", p=P))
                    for j in range(32):
                        g = q * 32 + j
                        b = int(meta["b_of"][g])
                        first = bool(meta["first"][g])
                        last = bool(meta["last"][g])
                        if first:
                            psb2 = psS.tile([P, G1], f32, tag="agg",
                                            name="agg2")
                            ps2[b] = psb2
                        psb2 = ps2[b]
                        stop = last and (not sched["g2b_nz"] if is_b else True)
                        nc.tensor.matmul(
                            psb2[:, :G2], lhsT=S2[:, j * P:(j + 1) * P],
                            rhs=gt2[:, j * G2:(j + 1) * G2],
                            start=first, stop=stop)
                        if not last:
                            continue
                        if not is_b:
                            nc.vector.tensor_copy(
                                agg2_s[:, b * G2:(b + 1) * G2],
                                ps2.pop(b)[:, :G2])
                        else:
                            l2_finalize(b)

            l2_round(L2A, 0, h2gA, False)
            l2_round(L2B, L2A["nblocks"] // 8, h2gB, True)

            # batched log_softmax over all node blocks (logits are tiny:
            # exp without max-shift is safe)
            e_all = qp.tile([P, NB * FOUT], f32, tag="eall", bufs=1)
            nc.scalar.activation(e_all[:, :], out_acc[:, :], AF.Exp)
            sums = qp.tile([P, NB], f32, tag="sums", bufs=1)
            nc.vector.reduce_sum(
                sums[:, :],
                e_all[:, :].rearrange("p (b f) -> p b f", f=FOUT),
                axis=AX.X)
            lns = qp.tile([P, NB], f32, tag="lns", bufs=1)
            nc.scalar.activation(lns[:, :], sums[:, :], AF.Ln)
            res = qp.tile([P, NB * FOUT], f32, tag="eall", bufs=1, name="res")
            nc.vector.tensor_tensor(
                out=res[:, :].rearrange("p (b f) -> p b f", f=FOUT),
                in0=out_acc[:, :].rearrange("p (b f) -> p b f", f=FOUT),
                in1=lns[:, :].unsqueeze(2).to_broadcast([P, NB, FOUT]),
                op=OP.subtract)
            nc.scalar.dma_start(
                out=out_d[:, :].rearrange("(b p) f -> p b f", p=P),
                in_=res[:, :].rearrange("p (b f) -> p b f", f=FOUT))
            if DBG:
                # sample blocks across h1g0/h1bR0 to identify AG layout
                probes = ([(h1gs[0], r) for r in
                           [0, 896, 1792, 2688, 3584, 7168, 10752, 14208]] +
                          [(h1bs[0], r) for r in [0, 448, 768]])
                for i, (tsrc, r0) in enumerate(probes):
                    tt = qp.tile([P, G1], f8, tag="dbgt", bufs=2)
                    nc.sync.dma_start(out=tt[:, :], in_=tsrc[r0:r0 + P, :])
                    nc.sync.dma_start(out=dbg_t[i * P:(i + 1) * P, :],
                                      in_=tt[:, :])
    return nc


_LAST_EXEC_NS = None
_LAST_RESULT = None


def run(inputs, cfg, trace=False, debug=False):
    global _LAST_EXEC_NS, _LAST_RESULT
    in_maps, sched = host_prep(inputs, cfg)
    nc = build(cfg, sched, debug=debug)
    nc.finalize()
    from concourse import bass_utils
    res = bass_utils.run_bass_kernel_spmd(
        nc, in_maps, core_ids=list(range(cfg.NC)), trace=trace)
    _LAST_EXEC_NS = res.exec_time_ns
    _LAST_RESULT = res
    outs = [np.asarray(res.results[c]["out"])[:cfg.NLOC_RAW]
            for c in range(cfg.NC)]
    return np.concatenate(outs, 0).astype(np.float32)


def kernel(**inputs):
    return run(inputs, _Cfg(**CFG_FULL))
